# revision 36
# baseline (speedup 1.0000x reference)
"""Multi-head causal attention (B=2, S=2048, E=1024, H=16, Dh=64) on 8 TRN2
NeuronCores.

Sharding: core c handles batch c//4 and the 4 heads [4*(c%4), 4*(c%4)+4).
Each core computes its heads' QKV projections, causal softmax attention, and
a partial output projection (contraction over its 256 d_inner columns).
The host sums the 4 partial outputs per batch (the "all-reduce") and adds
bo_eff = bo + bv @ Wo (the V bias commutes through softmax since the
attention weights sum to 1, so it is folded into the output bias on host).

Device layout notes (all matmul operands bf16, PSUM accumulation fp32):
  - Activations enter as X^T (E-major); Q,K are produced transposed
    (d-major [dl, s]) so the score matmuls contract dh over partitions.
  - Scores are computed transposed [k, q]; exp runs on the scalar engine
    (the only engine with an activation unit - it is the second-busiest
    resource after the PE and is kept free of everything else); the PV
    matmul is computed as AO[q, dh] = P^T V with keys contracting over
    partitions at full 128x128 PE utilization, with a ones column of V
    producing the softmax denominators.  AO is normalized per-partition on
    DVE and transposed back to [dh, q] by XBAR DMA transposes.
  - The causal staircase is trimmed at 128-column granularity on the
    diagonal k-tiles; only the leading [128,128] chunk of each needs a
    triangular mask multiply (gpsimd, which cannot touch PSUM).
  - Q/K/V projection chunks and output-projection tiles are issued as
    "filler" PE work inside the attention kb loops, balanced so the PE
    stream paces the exp stream; attention groups interleave head pairs
    (qb,hp) = (0,0),(1,0),(0,1),(2,0),(1,1),(3,0),(2,1),(3,1) so filler
    supply exists in every phase.
"""

import numpy as np
import ml_dtypes

import concourse.bass as bass
import concourse.tile as tile
from concourse import bacc, mybir
from concourse.bass_utils import run_bass_kernel_spmd

F32 = mybir.dt.float32
BF16 = mybir.dt.bfloat16
F16 = mybir.dt.float16

B, S, E = 2, 2048, 1024
H, DH = 16, 64
NCORES = 8
HPC = 4          # heads per core
DL = HPC * DH    # 256: d_inner slice per core
NKT = E // 128   # 8 k-tiles over embed dim
NST = S // 128   # 16 seq tiles of 128
NQB = S // 512   # 4 q blocks of 512

ExpF = mybir.ActivationFunctionType.Exp

NPBF16 = ml_dtypes.bfloat16


def build_nc():
    nc = bacc.Bacc("TRN2", target_bir_lowering=False)

    xt_d = nc.dram_tensor("xt", [E, S], BF16, kind="ExternalInput")
    wq_d = nc.dram_tensor("wq", [E, DL], BF16, kind="ExternalInput")
    wk_d = nc.dram_tensor("wk", [E, DL], BF16, kind="ExternalInput")
    wv_d = nc.dram_tensor("wv", [E, DL], BF16, kind="ExternalInput")
    wo_d = nc.dram_tensor("wo", [DL, E], BF16, kind="ExternalInput")
    bqk_d = nc.dram_tensor("bqk", [128, 4], F32, kind="ExternalInput")
    tri_d = nc.dram_tensor("tri", [128, 128], BF16, kind="ExternalInput")
    out_d = nc.dram_tensor("out", [E, S], F16, kind="ExternalOutput")

    with tile.TileContext(nc) as tc:
        with (
            tc.tile_pool(name="const", bufs=1) as cp,
            tc.tile_pool(name="work", bufs=1) as wkp,
            tc.tile_pool(name="bpsum", bufs=1, space="PSUM") as bp,
            tc.tile_pool(name="apsum", bufs=1, space="PSUM") as aop,
        ):
            xt_all = cp.tile([128, NKT, S], BF16, tag="xt", name="xt")
            xt = [xt_all[:, k, :] for k in range(NKT)]
            wq_sb = cp.tile([128, NKT, DL], BF16, tag="wq_sb", name="wq_sb")
            wk_sb = cp.tile([128, NKT, DL], BF16, tag="wk_sb", name="wk_sb")
            wv_sb = cp.tile([128, NKT, DL], BF16, tag="wv_sb", name="wv_sb")
            wo_sb = cp.tile([128, 2, E], BF16, tag="wo_sb", name="wo_sb")
            bqk = cp.tile([128, 4], F32, tag="bqk", name="bqk")
            tri = cp.tile([128, 128], BF16, tag="tri", name="tri")
            qt = [cp.tile([128, S], BF16, tag=f"qt{m}", name=f"qt{m}")
                  for m in range(2)]
            kt = [cp.tile([128, S], BF16, tag=f"kt{m}", name=f"kt{m}")
                  for m in range(2)]
            v1 = cp.tile([128, NST, HPC * 65], BF16, tag="v1", name="v1")
            ot = [cp.tile([128, S], BF16, tag=f"ot{d}", name=f"ot{d}")
                  for d in range(2)]

            # ---- input DMA stream (ordered for earliest compute start) ----
            xt_src = xt_d.rearrange("(k p) s -> p k s", p=128)
            nc.sync.dma_start(out=tri[:], in_=tri_d[:])
            nc.sync.dma_start(out=bqk[:], in_=bqk_d[:])
            nc.sync.dma_start(
                out=wq_sb[:], in_=wq_d.rearrange("(k p) c -> p k c", p=128))
            nc.sync.dma_start(out=xt_all[:, :, 0:512],
                              in_=xt_src[:, :, 0:512])
            nc.sync.dma_start(
                out=wk_sb[:], in_=wk_d.rearrange("(k p) c -> p k c", p=128))
            nc.sync.dma_start(
                out=wv_sb[:], in_=wv_d.rearrange("(k p) c -> p k c", p=128))
            for sb in range(1, 4):
                nc.sync.dma_start(
                    out=xt_all[:, :, sb * 512:(sb + 1) * 512],
                    in_=xt_src[:, :, sb * 512:(sb + 1) * 512])
            nc.sync.dma_start(
                out=wo_sb[:], in_=wo_d.rearrange("(d p) e -> p d e", p=128))

            # ---- PE p-state warm-up: ~3us of junk matmuls on tri during
            # the DMA lead-in so the real projections start at full rate
            warm = aop.tile([128, 4 * 65], F32, tag="ao0", bufs=1,
                            name="warm")
            for _ in range(48):
                nc.tensor.matmul(warm[:, 0:128], tri[:], tri[:],
                                 start=True, stop=True)

            # ones column of v1 (col 64 of each head's 65-col group)
            nc.gpsimd.memset(
                v1.rearrange("p s (h c) -> p s h c", c=65)[:, :, :, 64:65],
                1.0)

            # ---- projection building blocks ----
            def qk_chunk(mat, m, sb):
                """qt/kt[m][:, sb*512:+512] = (W.T X)^T chunk + bias."""
                w_sb = wq_sb if mat == 0 else wk_sb
                dst = (qt if mat == 0 else kt)[m]
                ps = bp.tile([128, 512], F32, tag="big", bufs=3, name="ps")
                for k in range(NKT):
                    nc.tensor.matmul(
                        ps[:],
                        w_sb[:, k, m * 128:(m + 1) * 128],
                        xt[k][:, sb * 512:(sb + 1) * 512],
                        start=(k == 0), stop=(k == NKT - 1))
                with nc.allow_low_precision(reason="bf16 round of q/k"):
                    nc.vector.tensor_scalar_add(
                        dst[:, sb * 512:(sb + 1) * 512], ps[:],
                        bqk[:, 2 * mat + m:2 * mat + m + 1])

            def v_chunk(st):
                """v1[:, st, 65h:65h+64] = (X Wv)[st*128:+128, 64h:+64]."""
                ps = bp.tile([128, DL], F32, tag="big", bufs=3, name="psv")
                for k in range(NKT):
                    nc.tensor.matmul(
                        ps[:],
                        xt[k][:, st * 128:(st + 1) * 128],
                        wv_sb[:, k, :],
                        start=(k == 0), stop=(k == NKT - 1))
                with nc.allow_low_precision(reason="bf16 round of v"):
                    nc.vector.tensor_copy(
                        v1[:, st, :].rearrange("p (h c) -> p h c",
                                               c=65)[:, :, 0:64],
                        ps[:].rearrange("p (h c) -> p h c", c=64))

            # ---- filler queue: PE work pumped into attention kb steps ----
            # Each entry: (tag, pe_ns_estimate, fn).  pump() keeps a credit
            # in ns: attention kb steps add their exp-vs-PE deficit, fillers
            # subtract their cost (credit may go negative and self-balance).
            fillers = []
            state = {"credit": 0.0}

            def pump(need_ns):
                state["credit"] += need_ns
                while fillers and state["credit"] > 0:
                    tag, ns, fn = fillers.pop(0)
                    state["credit"] = max(state["credit"] - ns, -1200.0)
                    fn()

            def drain(tags):
                # force-drains run work earlier than the credit schedule
                # would; they do not charge credit (PE-feeding beats exact
                # exp pacing, since total PE work exceeds total exp work)
                while any(t in tags for t, _, _ in fillers):
                    fillers.pop(0)[2]()

            def oproj_pair(qb, et, on_act):
                """out[et*128:(et+2)*128, qb*512:+512] partials (2 e-tiles)."""
                ob = wkp.tile([128, 1024], F16, tag="ob", bufs=4, name="ob")
                for i in range(2):
                    p3 = bp.tile([128, 512], F32, tag="big", bufs=3,
                                 name="p3")
                    for d in range(2):
                        nc.tensor.matmul(
                            p3[:],
                            wo_sb[:, d, (et + i) * 128:(et + i + 1) * 128],
                            ot[d][:, qb * 512:(qb + 1) * 512],
                            start=(d == 0), stop=(d == 1))
                    with nc.allow_low_precision(reason="fp16 partial out"):
                        if on_act and i == 1:
                            nc.scalar.copy(
                                out=ob[:, i * 512:(i + 1) * 512], in_=p3[:])
                        else:
                            nc.vector.tensor_copy(
                                ob[:, i * 512:(i + 1) * 512], p3[:])
                nc.sync.dma_start(
                    out=out_d[et * 128:(et + 2) * 128,
                              qb * 512:(qb + 1) * 512].rearrange(
                                  "(i p) s -> p i s", p=128),
                    in_=ob[:].rearrange("p (i s) -> p i s", s=512))

            # ---- attention group: 512-wide q block x 2 heads ----
            # Issues the scores/exp/PV stream for the group; the PV tail
            # flushes, normalization and ot transposes are returned as
            # deferred "closeout" fillers, pumped inside the next group's
            # kb loop so the exp stream never pauses at group boundaries.
            def attn_group(qb, hp, trail=4, norm_on_act=False):
                q0 = qb * 512
                nkb = 4 * qb + 4
                aoh = [aop.tile([128, 4 * 65], F32, tag=f"ao{h}", bufs=1,
                                name=f"ao{h}") for h in range(2)]
                pend = []

                def flush_one(in_loop=True):
                    kb, ptt, w, j = pend.pop(0)
                    if in_loop:
                        # PV needs v1[:, kb]; the previous group's closeout
                        # must precede this group's first PV (AO reuse).
                        drain({f"v{kb}", "co"})
                    else:
                        drain({f"v{kb}"})
                    j0 = max(j, 0)
                    for h in range(2):
                        lh = 2 * hp + h
                        for qsub in range(j0, 4):
                            off = h * 512 + (qsub - j0) * 128
                            # one accumulation group per AO bank: start=True
                            # zeroes the whole 2KB zero region, so only the
                            # first matmul into the bank may set it; PSUM
                            # zeroes lazily on each address's first write
                            nc.tensor.matmul(
                                aoh[h][:, qsub * 65:qsub * 65 + 65],
                                ptt[:, off:off + 128],
                                v1[:, kb, lh * 65:(lh + 1) * 65],
                                start=(kb == 0 and qsub == 0),
                                stop=(kb == nkb - 1 and qsub == 3))

                # scores read qt[hp] cols [q0, q0+512) at every kb, and
                # kt[hp] cols [kb*128, ...) progressively
                drain({f"qkQ{hp}s{qb}"})
                for kb in range(nkb):
                    drain({f"qkK{hp}s{kb // 4}"})
                    if kb == 2:
                        # previous group's closeout must be issued before
                        # any of this group's PV matmuls touch AO buffers
                        drain({"co"})
                    j = kb - 4 * qb
                    w = 512 if j < 0 else 512 - 128 * j
                    qs = q0 + (0 if j < 0 else 128 * j)
                    # head h occupies cols [h*512, h*512+w) so every matmul
                    # output stays inside one 2KB PSUM bank
                    st = bp.tile([128, 1024], F32, tag="big", bufs=3,
                                 name="st")
                    for h in range(2):
                        nc.tensor.matmul(
                            st[:, h * 512:h * 512 + w],
                            kt[hp][h * 64:(h + 1) * 64,
                                   kb * 128:(kb + 1) * 128],
                            qt[hp][h * 64:(h + 1) * 64, qs:q0 + 512],
                            start=True, stop=True)
                    ptt = wkp.tile([128, 1024], BF16, tag="pt", bufs=8,
                                   name="pt")
                    nc.scalar.activation(
                        ptt[:].rearrange("p (a b) -> p a b", a=2)[:, :, 0:w],
                        st[:].rearrange("p (a b) -> p a b", a=2)[:, :, 0:w],
                        ExpF, scale=0.125)
                    if j >= 0:
                        for h in range(2):
                            with nc.allow_low_precision(
                                    reason="0/1 mask multiply"):
                                nc.gpsimd.tensor_mul(
                                    ptt[:, h * 512:h * 512 + 128],
                                    ptt[:, h * 512:h * 512 + 128], tri[:])
                    pend.append((kb, ptt, w, j))
                    npv = 2 * (4 - max(j, 0))
                    if len(pend) > trail:
                        flush_one()
                    # deficit: exp time minus this kb's attention PE time
                    act_ns = 2 * w * 0.8333 + 220
                    pe_ns = 2 * w * 0.4167 + npv * 30
                    pump(act_ns - pe_ns)

                def do_norm():
                    # normalize into [q, dh-pair] SBUF tiles, then XBAR-DMA
                    # transpose each back into ot[hp][:, q block]
                    rc = wkp.tile([128, 8], F32, tag="rcp", bufs=2,
                                  name="rc")
                    for h in range(2):
                        with nc.allow_low_precision(
                                reason="softmax denom recip"):
                            nc.vector.reciprocal(
                                rc[:].rearrange("p (h q) -> p h q",
                                                h=2)[:, h, :].rearrange(
                                                    "p (q c) -> p q c", c=1),
                                aoh[h].rearrange("p (q c) -> p q c",
                                                 c=65)[:, :, 64:65])
                    for qsub in range(4):
                        asb = wkp.tile([128, 128], BF16, tag="aosb", bufs=8,
                                       name="asb")
                        for h in range(2):
                            with nc.allow_low_precision(
                                    reason="bf16 attn out"):
                                if norm_on_act:
                                    # post-exp-stream groups: Act is idle
                                    nc.scalar.mul(
                                        asb[:, h * 64:(h + 1) * 64],
                                        aoh[h][:, qsub * 65:qsub * 65 + 64],
                                        rc[:, h * 4 + qsub:h * 4 + qsub + 1])
                                else:
                                    nc.vector.tensor_scalar_mul(
                                        asb[:, h * 64:(h + 1) * 64],
                                        aoh[h][:, qsub * 65:qsub * 65 + 64],
                                        rc[:, h * 4 + qsub:h * 4 + qsub + 1])
                        nc.sync.dma_start_transpose(
                            ot[hp][:, q0 + qsub * 128:q0 + (qsub + 1) * 128],
                            asb[:])

                ntail = len(pend)

                def do_closeout():
                    while pend:
                        flush_one(in_loop=False)
                    do_norm()

                return ("co", ntail * 240.0, do_closeout)

            # ---- schedule ----
            # upfront: just enough projection to start attention (0,0);
            # everything else becomes filler, force-drained on first use
            qk_chunk(0, 0, 0)
            qk_chunk(1, 0, 0)

            QK_NS = 8 * 512 * 0.4167
            V_NS = 8 * 256 * 0.4167
            OP_NS = 4 * 512 * 0.4167

            def add_qk(m, sb):
                fillers.append((f"qkQ{m}s{sb}", QK_NS,
                                lambda sb=sb, m=m: qk_chunk(0, m, sb)))
                fillers.append((f"qkK{m}s{sb}", QK_NS,
                                lambda sb=sb, m=m: qk_chunk(1, m, sb)))

            def add_v(lo, hi):
                for st in range(lo, hi):
                    fillers.append((f"v{st}", V_NS,
                                    lambda st=st: v_chunk(st)))

            # deadline order: each entry no later than its force-drain point
            add_qk(0, 1)
            add_v(0, 4)
            add_qk(0, 2)
            add_v(4, 8)
            add_qk(1, 0)
            add_v(8, 12)
            add_qk(0, 3)
            add_v(12, 16)
            add_qk(1, 1)
            add_qk(1, 2)
            add_qk(1, 3)

            groups = [(0, 0), (1, 0), (2, 0), (3, 0), (0, 1), (1, 1),
                      (2, 1), (3, 1)]
            for gi, (qb, hp) in enumerate(groups):
                co = attn_group(qb, hp, trail=2 if gi == 7 else 4,
                                norm_on_act=(gi == 7))
                fillers.insert(0, co)
                if hp == 1:
                    on_act = gi >= 7  # exp stream done; Act is free
                    for et in range(0, NKT, 2):
                        fillers.append(
                            (f"op{qb}", OP_NS,
                             lambda qb=qb, et=et, a=on_act:
                             oproj_pair(qb, et, a)))
            drain({t for t, _, _ in fillers})

    nc.compile()
    return nc


_NC = None


def _get_nc():
    global _NC
    if _NC is None:
        _NC = build_nc()
    return _NC


def make_in_maps(inputs, Wq, bq, Wk, bk, Wv, Wo):
    kk = np.arange(128)[:, None]
    qq = np.arange(128)[None, :]
    tri = (qq >= kk).astype(NPBF16)
    in_maps = []
    for c in range(NCORES):
        b, g = c // HPC, c % HPC
        sl = slice(g * DL, (g + 1) * DL)
        bqk = np.stack([bq[sl][:128], bq[sl][128:],
                        bk[sl][:128], bk[sl][128:]], axis=1)
        in_maps.append({
            "xt": np.ascontiguousarray(inputs[b].T).astype(NPBF16),
            "wq": np.ascontiguousarray(Wq[:, sl]).astype(NPBF16),
            "wk": np.ascontiguousarray(Wk[:, sl]).astype(NPBF16),
            "wv": np.ascontiguousarray(Wv[:, sl]).astype(NPBF16),
            "wo": np.ascontiguousarray(Wo[sl, :]).astype(NPBF16),
            "bqk": np.ascontiguousarray(bqk).astype(np.float32),
            "tri": tri,
        })
    return in_maps


def kernel(inputs, Wq, bq, Wk, bk, Wv, bv, Wo, bo):
    inputs = np.asarray(inputs, np.float32)
    Wq, bq, Wk, bk, Wv, bv, Wo, bo = (
        np.asarray(a, np.float32) for a in (Wq, bq, Wk, bk, Wv, bv, Wo, bo))
    in_maps = make_in_maps(inputs, Wq, bq, Wk, bk, Wv, Wo)
    nc = _get_nc()
    res = run_bass_kernel_spmd(nc, in_maps, list(range(NCORES)))
    bo_eff = bo + bv @ Wo  # V bias commutes through softmax (weights sum to 1)
    outs = []
    for b in range(B):
        acc = res.results[b * HPC]["out"].astype(np.float32)
        for g in range(1, HPC):
            acc = acc + res.results[b * HPC + g]["out"].astype(np.float32)
        outs.append(acc.T + bo_eff)
    return np.stack(outs).astype(np.float32)


# revision 37
# speedup vs baseline: 1.0076x; 1.0076x over previous
"""Multi-head causal attention (B=2, S=2048, E=1024, H=16, Dh=64) on 8 TRN2
NeuronCores.

Sharding: core c handles batch c//4 and the 4 heads [4*(c%4), 4*(c%4)+4).
Each core computes its heads' QKV projections, causal softmax attention, and
a partial output projection (contraction over its 256 d_inner columns).
The host sums the 4 partial outputs per batch (the "all-reduce") and adds
bo_eff = bo + bv @ Wo (the V bias commutes through softmax since the
attention weights sum to 1, so it is folded into the output bias on host).

Device layout notes (all matmul operands bf16, PSUM accumulation fp32):
  - Activations enter as X^T (E-major); Q,K are produced transposed
    (d-major [dl, s]) so the score matmuls contract dh over partitions.
  - Scores are computed transposed [k, q]; exp runs on the scalar engine
    (the only engine with an activation unit - it is the second-busiest
    resource after the PE and is kept free of everything else); the PV
    matmul is computed as AO[q, dh] = P^T V with keys contracting over
    partitions at full 128x128 PE utilization, with a ones column of V
    producing the softmax denominators.  AO is normalized per-partition on
    DVE and transposed back to [dh, q] by XBAR DMA transposes.
  - The causal staircase is trimmed at 128-column granularity on the
    diagonal k-tiles; only the leading [128,128] chunk of each needs a
    triangular mask multiply (gpsimd, which cannot touch PSUM).
  - Q/K/V projection chunks and output-projection tiles are issued as
    "filler" PE work inside the attention kb loops, balanced so the PE
    stream paces the exp stream; attention groups interleave head pairs
    (qb,hp) = (0,0),(1,0),(0,1),(2,0),(1,1),(3,0),(2,1),(3,1) so filler
    supply exists in every phase.
"""

import numpy as np
import ml_dtypes

import concourse.bass as bass
import concourse.tile as tile
from concourse import bacc, mybir
from concourse.bass_utils import run_bass_kernel_spmd

F32 = mybir.dt.float32
BF16 = mybir.dt.bfloat16
F16 = mybir.dt.float16

B, S, E = 2, 2048, 1024
H, DH = 16, 64
NCORES = 8
HPC = 4          # heads per core
DL = HPC * DH    # 256: d_inner slice per core
NKT = E // 128   # 8 k-tiles over embed dim
NST = S // 128   # 16 seq tiles of 128
NQB = S // 512   # 4 q blocks of 512

ExpF = mybir.ActivationFunctionType.Exp

NPBF16 = ml_dtypes.bfloat16


def build_nc():
    nc = bacc.Bacc("TRN2", target_bir_lowering=False)

    xt_d = nc.dram_tensor("xt", [E, S], BF16, kind="ExternalInput")
    wq_d = nc.dram_tensor("wq", [E, DL], BF16, kind="ExternalInput")
    wk_d = nc.dram_tensor("wk", [E, DL], BF16, kind="ExternalInput")
    wv_d = nc.dram_tensor("wv", [E, DL], BF16, kind="ExternalInput")
    wo_d = nc.dram_tensor("wo", [DL, E], BF16, kind="ExternalInput")
    bqk_d = nc.dram_tensor("bqk", [128, 4], F32, kind="ExternalInput")
    tri_d = nc.dram_tensor("tri", [128, 128], BF16, kind="ExternalInput")
    out_d = nc.dram_tensor("out", [E, S], F16, kind="ExternalOutput")

    with tile.TileContext(nc) as tc:
        with (
            tc.tile_pool(name="const", bufs=1) as cp,
            tc.tile_pool(name="work", bufs=1) as wkp,
            tc.tile_pool(name="bpsum", bufs=1, space="PSUM") as bp,
            tc.tile_pool(name="apsum", bufs=1, space="PSUM") as aop,
        ):
            xt_all = cp.tile([128, NKT, S], BF16, tag="xt", name="xt")
            xt = [xt_all[:, k, :] for k in range(NKT)]
            wq_sb = cp.tile([128, NKT, DL], BF16, tag="wq_sb", name="wq_sb")
            wk_sb = cp.tile([128, NKT, DL], BF16, tag="wk_sb", name="wk_sb")
            wv_sb = cp.tile([128, NKT, DL], BF16, tag="wv_sb", name="wv_sb")
            wo_sb = cp.tile([128, 2, E], BF16, tag="wo_sb", name="wo_sb")
            bqk = cp.tile([128, 4], F32, tag="bqk", name="bqk")
            tri = cp.tile([128, 128], BF16, tag="tri", name="tri")
            qt = [cp.tile([128, S], BF16, tag=f"qt{m}", name=f"qt{m}")
                  for m in range(2)]
            kt = [cp.tile([128, S], BF16, tag=f"kt{m}", name=f"kt{m}")
                  for m in range(2)]
            v1 = cp.tile([128, NST, HPC * 65], BF16, tag="v1", name="v1")
            ot = [cp.tile([128, S], BF16, tag=f"ot{d}", name=f"ot{d}")
                  for d in range(2)]

            # ---- input DMA stream (ordered for earliest compute start) ----
            xt_src = xt_d.rearrange("(k p) s -> p k s", p=128)
            nc.sync.dma_start(out=tri[:], in_=tri_d[:])
            nc.sync.dma_start(out=bqk[:], in_=bqk_d[:])
            nc.sync.dma_start(
                out=wq_sb[:], in_=wq_d.rearrange("(k p) c -> p k c", p=128))
            nc.sync.dma_start(out=xt_all[:, :, 0:512],
                              in_=xt_src[:, :, 0:512])
            nc.sync.dma_start(
                out=wk_sb[:], in_=wk_d.rearrange("(k p) c -> p k c", p=128))
            nc.sync.dma_start(
                out=wv_sb[:], in_=wv_d.rearrange("(k p) c -> p k c", p=128))
            for sb in range(1, 4):
                nc.sync.dma_start(
                    out=xt_all[:, :, sb * 512:(sb + 1) * 512],
                    in_=xt_src[:, :, sb * 512:(sb + 1) * 512])
            nc.sync.dma_start(
                out=wo_sb[:], in_=wo_d.rearrange("(d p) e -> p d e", p=128))

            # ---- PE p-state warm-up: ~3us of junk matmuls on tri during
            # the DMA lead-in so the real projections start at full rate
            warm = aop.tile([128, 4 * 65], F32, tag="ao0", bufs=1,
                            name="warm")
            for _ in range(0):
                nc.tensor.matmul(warm[:, 0:128], tri[:], tri[:],
                                 start=True, stop=True)

            # ones column of v1 (col 64 of each head's 65-col group)
            nc.gpsimd.memset(
                v1.rearrange("p s (h c) -> p s h c", c=65)[:, :, :, 64:65],
                1.0)

            # ---- projection building blocks ----
            def qk_chunk(mat, m, sb):
                """qt/kt[m][:, sb*512:+512] = (W.T X)^T chunk + bias."""
                w_sb = wq_sb if mat == 0 else wk_sb
                dst = (qt if mat == 0 else kt)[m]
                ps = bp.tile([128, 512], F32, tag="big", bufs=3, name="ps")
                for k in range(NKT):
                    nc.tensor.matmul(
                        ps[:],
                        w_sb[:, k, m * 128:(m + 1) * 128],
                        xt[k][:, sb * 512:(sb + 1) * 512],
                        start=(k == 0), stop=(k == NKT - 1))
                with nc.allow_low_precision(reason="bf16 round of q/k"):
                    nc.vector.tensor_scalar_add(
                        dst[:, sb * 512:(sb + 1) * 512], ps[:],
                        bqk[:, 2 * mat + m:2 * mat + m + 1])

            def v_chunk(st):
                """v1[:, st, 65h:65h+64] = (X Wv)[st*128:+128, 64h:+64]."""
                ps = bp.tile([128, DL], F32, tag="big", bufs=3, name="psv")
                for k in range(NKT):
                    nc.tensor.matmul(
                        ps[:],
                        xt[k][:, st * 128:(st + 1) * 128],
                        wv_sb[:, k, :],
                        start=(k == 0), stop=(k == NKT - 1))
                with nc.allow_low_precision(reason="bf16 round of v"):
                    nc.vector.tensor_copy(
                        v1[:, st, :].rearrange("p (h c) -> p h c",
                                               c=65)[:, :, 0:64],
                        ps[:].rearrange("p (h c) -> p h c", c=64))

            # ---- filler queue: PE work pumped into attention kb steps ----
            # Each entry: (tag, pe_ns_estimate, fn).  pump() keeps a credit
            # in ns: attention kb steps add their exp-vs-PE deficit, fillers
            # subtract their cost (credit may go negative and self-balance).
            fillers = []
            state = {"credit": 0.0}

            def pump(need_ns):
                state["credit"] += need_ns
                while fillers and state["credit"] > 0:
                    tag, ns, fn = fillers.pop(0)
                    state["credit"] = max(state["credit"] - ns, -1200.0)
                    fn()

            def drain(tags):
                # force-drains run work earlier than the credit schedule
                # would; they do not charge credit (PE-feeding beats exact
                # exp pacing, since total PE work exceeds total exp work)
                while any(t in tags for t, _, _ in fillers):
                    fillers.pop(0)[2]()

            def oproj_pair(qb, et, on_act):
                """out[et*128:(et+2)*128, qb*512:+512] partials (2 e-tiles)."""
                ob = wkp.tile([128, 1024], F16, tag="ob", bufs=4, name="ob")
                for i in range(2):
                    p3 = bp.tile([128, 512], F32, tag="big", bufs=3,
                                 name="p3")
                    for d in range(2):
                        nc.tensor.matmul(
                            p3[:],
                            wo_sb[:, d, (et + i) * 128:(et + i + 1) * 128],
                            ot[d][:, qb * 512:(qb + 1) * 512],
                            start=(d == 0), stop=(d == 1))
                    with nc.allow_low_precision(reason="fp16 partial out"):
                        if on_act and i == 1:
                            nc.scalar.copy(
                                out=ob[:, i * 512:(i + 1) * 512], in_=p3[:])
                        else:
                            nc.vector.tensor_copy(
                                ob[:, i * 512:(i + 1) * 512], p3[:])
                nc.sync.dma_start(
                    out=out_d[et * 128:(et + 2) * 128,
                              qb * 512:(qb + 1) * 512].rearrange(
                                  "(i p) s -> p i s", p=128),
                    in_=ob[:].rearrange("p (i s) -> p i s", s=512))

            # ---- attention group: 512-wide q block x 2 heads ----
            # Issues the scores/exp/PV stream for the group; the PV tail
            # flushes, normalization and ot transposes are returned as
            # deferred "closeout" fillers, pumped inside the next group's
            # kb loop so the exp stream never pauses at group boundaries.
            def attn_group(qb, hp, trail=4, norm_on_act=False):
                q0 = qb * 512
                nkb = 4 * qb + 4
                aoh = [aop.tile([128, 4 * 65], F32, tag=f"ao{h}", bufs=1,
                                name=f"ao{h}") for h in range(2)]
                pend = []

                def flush_one(in_loop=True):
                    kb, ptt, w, j = pend.pop(0)
                    if in_loop:
                        # PV needs v1[:, kb]; the previous group's closeout
                        # must precede this group's first PV (AO reuse).
                        drain({f"v{kb}", "co"})
                    else:
                        drain({f"v{kb}"})
                    j0 = max(j, 0)
                    for h in range(2):
                        lh = 2 * hp + h
                        for qsub in range(j0, 4):
                            off = h * 512 + (qsub - j0) * 128
                            # one accumulation group per AO bank: start=True
                            # zeroes the whole 2KB zero region, so only the
                            # first matmul into the bank may set it; PSUM
                            # zeroes lazily on each address's first write
                            nc.tensor.matmul(
                                aoh[h][:, qsub * 65:qsub * 65 + 65],
                                ptt[:, off:off + 128],
                                v1[:, kb, lh * 65:(lh + 1) * 65],
                                start=(kb == 0 and qsub == 0),
                                stop=(kb == nkb - 1 and qsub == 3))

                # scores read qt[hp] cols [q0, q0+512) at every kb, and
                # kt[hp] cols [kb*128, ...) progressively
                drain({f"qkQ{hp}s{qb}"})
                for kb in range(nkb):
                    drain({f"qkK{hp}s{kb // 4}"})
                    if kb == 2:
                        # previous group's closeout must be issued before
                        # any of this group's PV matmuls touch AO buffers
                        drain({"co"})
                    j = kb - 4 * qb
                    w = 512 if j < 0 else 512 - 128 * j
                    qs = q0 + (0 if j < 0 else 128 * j)
                    # head h occupies cols [h*512, h*512+w) so every matmul
                    # output stays inside one 2KB PSUM bank
                    st = bp.tile([128, 1024], F32, tag="big", bufs=3,
                                 name="st")
                    for h in range(2):
                        nc.tensor.matmul(
                            st[:, h * 512:h * 512 + w],
                            kt[hp][h * 64:(h + 1) * 64,
                                   kb * 128:(kb + 1) * 128],
                            qt[hp][h * 64:(h + 1) * 64, qs:q0 + 512],
                            start=True, stop=True)
                    ptt = wkp.tile([128, 1024], BF16, tag="pt", bufs=8,
                                   name="pt")
                    nc.scalar.activation(
                        ptt[:].rearrange("p (a b) -> p a b", a=2)[:, :, 0:w],
                        st[:].rearrange("p (a b) -> p a b", a=2)[:, :, 0:w],
                        ExpF, scale=0.125)
                    if j >= 0:
                        for h in range(2):
                            with nc.allow_low_precision(
                                    reason="0/1 mask multiply"):
                                nc.gpsimd.tensor_mul(
                                    ptt[:, h * 512:h * 512 + 128],
                                    ptt[:, h * 512:h * 512 + 128], tri[:])
                    pend.append((kb, ptt, w, j))
                    npv = 2 * (4 - max(j, 0))
                    if len(pend) > trail:
                        flush_one()
                    # deficit: exp time minus this kb's attention PE time
                    act_ns = 2 * w * 0.8333 + 220
                    pe_ns = 2 * w * 0.4167 + npv * 30
                    pump(act_ns - pe_ns)

                def do_norm():
                    # normalize into [q, dh-pair] SBUF tiles, then XBAR-DMA
                    # transpose each back into ot[hp][:, q block]
                    rc = wkp.tile([128, 8], F32, tag="rcp", bufs=2,
                                  name="rc")
                    for h in range(2):
                        with nc.allow_low_precision(
                                reason="softmax denom recip"):
                            nc.vector.reciprocal(
                                rc[:].rearrange("p (h q) -> p h q",
                                                h=2)[:, h, :].rearrange(
                                                    "p (q c) -> p q c", c=1),
                                aoh[h].rearrange("p (q c) -> p q c",
                                                 c=65)[:, :, 64:65])
                    for qsub in range(4):
                        asb = wkp.tile([128, 128], BF16, tag="aosb", bufs=8,
                                       name="asb")
                        for h in range(2):
                            with nc.allow_low_precision(
                                    reason="bf16 attn out"):
                                if norm_on_act:
                                    # post-exp-stream groups: Act is idle
                                    nc.scalar.mul(
                                        asb[:, h * 64:(h + 1) * 64],
                                        aoh[h][:, qsub * 65:qsub * 65 + 64],
                                        rc[:, h * 4 + qsub:h * 4 + qsub + 1])
                                else:
                                    nc.vector.tensor_scalar_mul(
                                        asb[:, h * 64:(h + 1) * 64],
                                        aoh[h][:, qsub * 65:qsub * 65 + 64],
                                        rc[:, h * 4 + qsub:h * 4 + qsub + 1])
                        nc.sync.dma_start_transpose(
                            ot[hp][:, q0 + qsub * 128:q0 + (qsub + 1) * 128],
                            asb[:])

                ntail = len(pend)

                def do_closeout():
                    while pend:
                        flush_one(in_loop=False)
                    do_norm()

                return ("co", ntail * 240.0, do_closeout)

            # ---- schedule ----
            # upfront: just enough projection to start attention (0,0);
            # everything else becomes filler, force-drained on first use
            qk_chunk(0, 0, 0)
            qk_chunk(1, 0, 0)

            QK_NS = 8 * 512 * 0.4167
            V_NS = 8 * 256 * 0.4167
            OP_NS = 4 * 512 * 0.4167

            def add_qk(m, sb):
                fillers.append((f"qkQ{m}s{sb}", QK_NS,
                                lambda sb=sb, m=m: qk_chunk(0, m, sb)))
                fillers.append((f"qkK{m}s{sb}", QK_NS,
                                lambda sb=sb, m=m: qk_chunk(1, m, sb)))

            def add_v(lo, hi):
                for st in range(lo, hi):
                    fillers.append((f"v{st}", V_NS,
                                    lambda st=st: v_chunk(st)))

            # deadline order: each entry no later than its force-drain point
            add_qk(0, 1)
            add_v(0, 4)
            add_qk(0, 2)
            add_v(4, 8)
            add_qk(1, 0)
            add_v(8, 12)
            add_qk(0, 3)
            add_v(12, 16)
            add_qk(1, 1)
            add_qk(1, 2)
            add_qk(1, 3)

            groups = [(0, 0), (1, 0), (2, 0), (3, 0), (0, 1), (1, 1),
                      (2, 1), (3, 1)]
            for gi, (qb, hp) in enumerate(groups):
                co = attn_group(qb, hp, trail=2 if gi == 7 else 4,
                                norm_on_act=False)
                fillers.insert(0, co)
                if hp == 1:
                    on_act = gi >= 7  # exp stream done; Act is free
                    for et in range(0, NKT, 2):
                        fillers.append(
                            (f"op{qb}", OP_NS,
                             lambda qb=qb, et=et, a=on_act:
                             oproj_pair(qb, et, a)))
            drain({t for t, _, _ in fillers})

    nc.compile()
    return nc


_NC = None


def _get_nc():
    global _NC
    if _NC is None:
        _NC = build_nc()
    return _NC


def make_in_maps(inputs, Wq, bq, Wk, bk, Wv, Wo):
    kk = np.arange(128)[:, None]
    qq = np.arange(128)[None, :]
    tri = (qq >= kk).astype(NPBF16)
    in_maps = []
    for c in range(NCORES):
        b, g = c // HPC, c % HPC
        sl = slice(g * DL, (g + 1) * DL)
        bqk = np.stack([bq[sl][:128], bq[sl][128:],
                        bk[sl][:128], bk[sl][128:]], axis=1)
        in_maps.append({
            "xt": np.ascontiguousarray(inputs[b].T).astype(NPBF16),
            "wq": np.ascontiguousarray(Wq[:, sl]).astype(NPBF16),
            "wk": np.ascontiguousarray(Wk[:, sl]).astype(NPBF16),
            "wv": np.ascontiguousarray(Wv[:, sl]).astype(NPBF16),
            "wo": np.ascontiguousarray(Wo[sl, :]).astype(NPBF16),
            "bqk": np.ascontiguousarray(bqk).astype(np.float32),
            "tri": tri,
        })
    return in_maps


def kernel(inputs, Wq, bq, Wk, bk, Wv, bv, Wo, bo):
    inputs = np.asarray(inputs, np.float32)
    Wq, bq, Wk, bk, Wv, bv, Wo, bo = (
        np.asarray(a, np.float32) for a in (Wq, bq, Wk, bk, Wv, bv, Wo, bo))
    in_maps = make_in_maps(inputs, Wq, bq, Wk, bk, Wv, Wo)
    nc = _get_nc()
    res = run_bass_kernel_spmd(nc, in_maps, list(range(NCORES)))
    bo_eff = bo + bv @ Wo  # V bias commutes through softmax (weights sum to 1)
    outs = []
    for b in range(B):
        acc = res.results[b * HPC]["out"].astype(np.float32)
        for g in range(1, HPC):
            acc = acc + res.results[b * HPC + g]["out"].astype(np.float32)
        outs.append(acc.T + bo_eff)
    return np.stack(outs).astype(np.float32)


# revision 40
# speedup vs baseline: 1.0609x; 1.0529x over previous
"""Multi-head causal attention (B=2, S=2048, E=1024, H=16, Dh=64) on 8 TRN2
NeuronCores.

Sharding: core c handles batch c//4 and the 4 heads [4*(c%4), 4*(c%4)+4).
Each core computes its heads' QKV projections, causal softmax attention, and
a partial output projection (contraction over its 256 d_inner columns).
The host sums the 4 partial outputs per batch (the "all-reduce") and adds
bo_eff = bo + bv @ Wo (the V bias commutes through softmax since the
attention weights sum to 1, so it is folded into the output bias on host).

Device layout notes (all matmul operands bf16, PSUM accumulation fp32):
  - Activations enter as X^T (E-major); Q,K are produced transposed
    (d-major [dl, s]) so the score matmuls contract dh over partitions.
  - Scores are computed transposed [k, q]; exp runs on the scalar engine
    (the only engine with an activation unit - it is the second-busiest
    resource after the PE and is kept free of everything else); the PV
    matmul is computed as AO[q, dh] = P^T V with keys contracting over
    partitions at full 128x128 PE utilization, with a ones column of V
    producing the softmax denominators.  AO is normalized per-partition on
    DVE and transposed back to [dh, q] by XBAR DMA transposes.
  - The causal staircase is trimmed at 128-column granularity on the
    diagonal k-tiles; only the leading [128,128] chunk of each needs a
    triangular mask multiply (gpsimd, which cannot touch PSUM).
  - Q/K/V projection chunks and output-projection tiles are issued as
    "filler" PE work inside the attention kb loops, balanced so the PE
    stream paces the exp stream; attention groups interleave head pairs
    (qb,hp) = (0,0),(1,0),(0,1),(2,0),(1,1),(3,0),(2,1),(3,1) so filler
    supply exists in every phase.
"""

import numpy as np
import ml_dtypes

import concourse.bass as bass
import concourse.tile as tile
from concourse import bacc, mybir
from concourse.bass_utils import run_bass_kernel_spmd

F32 = mybir.dt.float32
BF16 = mybir.dt.bfloat16
F16 = mybir.dt.float16

B, S, E = 2, 2048, 1024
H, DH = 16, 64
NCORES = 8
HPC = 4          # heads per core
DL = HPC * DH    # 256: d_inner slice per core
NKT = E // 128   # 8 k-tiles over embed dim
NST = S // 128   # 16 seq tiles of 128
NQB = S // 512   # 4 q blocks of 512

ExpF = mybir.ActivationFunctionType.Exp

NPBF16 = ml_dtypes.bfloat16


def build_nc():
    nc = bacc.Bacc("TRN2", target_bir_lowering=False)

    xt_d = nc.dram_tensor("xt", [E, S], BF16, kind="ExternalInput")
    wq_d = nc.dram_tensor("wq", [E, DL], BF16, kind="ExternalInput")
    wk_d = nc.dram_tensor("wk", [E, DL], BF16, kind="ExternalInput")
    wv_d = nc.dram_tensor("wv", [E, DL], BF16, kind="ExternalInput")
    wo_d = nc.dram_tensor("wo", [DL, E], BF16, kind="ExternalInput")
    bqk_d = nc.dram_tensor("bqk", [128, 4], F32, kind="ExternalInput")
    tri_d = nc.dram_tensor("tri", [128, 128], BF16, kind="ExternalInput")
    out_d = nc.dram_tensor("out", [E, S], F16, kind="ExternalOutput")

    with tile.TileContext(nc) as tc:
        with (
            tc.tile_pool(name="const", bufs=1) as cp,
            tc.tile_pool(name="work", bufs=1) as wkp,
            tc.tile_pool(name="bpsum", bufs=1, space="PSUM") as bp,
            tc.tile_pool(name="apsum", bufs=1, space="PSUM") as aop,
        ):
            xt_all = cp.tile([128, NKT, S], BF16, tag="xt", name="xt")
            xt = [xt_all[:, k, :] for k in range(NKT)]
            wq_sb = cp.tile([128, NKT, DL], BF16, tag="wq_sb", name="wq_sb")
            wk_sb = cp.tile([128, NKT, DL], BF16, tag="wk_sb", name="wk_sb")
            wv_sb = cp.tile([128, NKT, DL], BF16, tag="wv_sb", name="wv_sb")
            wo_sb = cp.tile([128, 2, E], BF16, tag="wo_sb", name="wo_sb")
            bqk = cp.tile([128, 4], F32, tag="bqk", name="bqk")
            tri = cp.tile([128, 128], BF16, tag="tri", name="tri")
            qt = [cp.tile([128, S], BF16, tag=f"qt{m}", name=f"qt{m}")
                  for m in range(2)]
            kt = [cp.tile([128, S], BF16, tag=f"kt{m}", name=f"kt{m}")
                  for m in range(2)]
            v1 = cp.tile([128, NST, HPC * 65], BF16, tag="v1", name="v1")
            ot = [cp.tile([128, S], BF16, tag=f"ot{d}", name=f"ot{d}")
                  for d in range(2)]

            # ---- input DMA stream (ordered for earliest compute start) ----
            nc.sync.dma_start(out=bqk[:], in_=bqk_d[:])
            nc.sync.dma_start(
                out=wq_sb[:], in_=wq_d.rearrange("(k p) c -> p k c", p=128))
            for k in range(4):
                nc.sync.dma_start(
                    out=xt[k][:, 0:512], in_=xt_d[k * 128:(k + 1) * 128, 0:512])
            nc.sync.dma_start(
                out=wk_sb[:], in_=wk_d.rearrange("(k p) c -> p k c", p=128))
            for k in range(4, NKT):
                nc.sync.dma_start(
                    out=xt[k][:, 0:512], in_=xt_d[k * 128:(k + 1) * 128, 0:512])
            nc.sync.dma_start(
                out=wv_sb[:], in_=wv_d.rearrange("(k p) c -> p k c", p=128))
            nc.sync.dma_start(out=tri[:], in_=tri_d[:])
            for k in range(NKT):
                nc.sync.dma_start(
                    out=xt[k][:, 512:1024],
                    in_=xt_d[k * 128:(k + 1) * 128, 512:1024])
            for k in range(NKT):
                nc.sync.dma_start(
                    out=xt[k][:, 1024:2048],
                    in_=xt_d[k * 128:(k + 1) * 128, 1024:2048])
            nc.sync.dma_start(
                out=wo_sb[:], in_=wo_d.rearrange("(d p) e -> p d e", p=128))

            # ones column of v1 (col 64 of each head's 65-col group)
            nc.gpsimd.memset(
                v1.rearrange("p s (h c) -> p s h c", c=65)[:, :, :, 64:65],
                1.0)

            # ---- projection building blocks ----
            def qk_chunk(mat, m, sb):
                """qt/kt[m][:, sb*512:+512] = (W.T X)^T chunk + bias."""
                w_sb = wq_sb if mat == 0 else wk_sb
                dst = (qt if mat == 0 else kt)[m]
                ps = bp.tile([128, 512], F32, tag="big", bufs=3, name="ps")
                for k in range(NKT):
                    nc.tensor.matmul(
                        ps[:],
                        w_sb[:, k, m * 128:(m + 1) * 128],
                        xt[k][:, sb * 512:(sb + 1) * 512],
                        start=(k == 0), stop=(k == NKT - 1))
                with nc.allow_low_precision(reason="bf16 round of q/k"):
                    nc.vector.tensor_scalar_add(
                        dst[:, sb * 512:(sb + 1) * 512], ps[:],
                        bqk[:, 2 * mat + m:2 * mat + m + 1])

            def v_chunk(st):
                """v1[:, st, 65h:65h+64] = (X Wv)[st*128:+128, 64h:+64]."""
                ps = bp.tile([128, DL], F32, tag="big", bufs=3, name="psv")
                for k in range(NKT):
                    nc.tensor.matmul(
                        ps[:],
                        xt[k][:, st * 128:(st + 1) * 128],
                        wv_sb[:, k, :],
                        start=(k == 0), stop=(k == NKT - 1))
                with nc.allow_low_precision(reason="bf16 round of v"):
                    nc.vector.tensor_copy(
                        v1[:, st, :].rearrange("p (h c) -> p h c",
                                               c=65)[:, :, 0:64],
                        ps[:].rearrange("p (h c) -> p h c", c=64))

            # ---- filler queue: PE work pumped into attention kb steps ----
            # Each entry: (tag, pe_ns_estimate, fn).  pump() keeps a credit
            # in ns: attention kb steps add their exp-vs-PE deficit, fillers
            # subtract their cost (credit may go negative and self-balance).
            fillers = []
            state = {"credit": 0.0}

            def pump(need_ns):
                state["credit"] += need_ns
                while fillers and state["credit"] > 0:
                    tag, ns, fn = fillers.pop(0)
                    state["credit"] = max(state["credit"] - ns, -1200.0)
                    fn()

            def drain(tags):
                # force-drains run work earlier than the credit schedule
                # would; they do not charge credit (PE-feeding beats exact
                # exp pacing, since total PE work exceeds total exp work)
                while any(t in tags for t, _, _ in fillers):
                    fillers.pop(0)[2]()

            def oproj_pair(qb, et, on_act):
                """out[et*128:(et+2)*128, qb*512:+512] partials (2 e-tiles)."""
                ob = wkp.tile([128, 1024], F16, tag="ob", bufs=4, name="ob")
                p3 = bp.tile([128, 1024], F32, tag="big", bufs=3, name="p3")
                for i in range(2):
                    for d in range(2):
                        nc.tensor.matmul(
                            p3[:, i * 512:(i + 1) * 512],
                            wo_sb[:, d, (et + i) * 128:(et + i + 1) * 128],
                            ot[d][:, qb * 512:(qb + 1) * 512],
                            start=(d == 0), stop=(d == 1))
                with nc.allow_low_precision(reason="fp16 partial out"):
                    if on_act:
                        nc.scalar.copy(out=ob[:], in_=p3[:])
                    else:
                        nc.vector.tensor_copy(ob[:], p3[:])
                nc.sync.dma_start(
                    out=out_d[et * 128:(et + 2) * 128,
                              qb * 512:(qb + 1) * 512].rearrange(
                                  "(i p) s -> p i s", p=128),
                    in_=ob[:].rearrange("p (i s) -> p i s", s=512))

            # ---- attention group: 512-wide q block x 2 heads ----
            # Issues the scores/exp/PV stream for the group; the PV tail
            # flushes, normalization and ot transposes are returned as
            # deferred "closeout" fillers, pumped inside the next group's
            # kb loop so the exp stream never pauses at group boundaries.
            def attn_group(qb, hp, trail=4, norm_on_act=False):
                q0 = qb * 512
                nkb = 4 * qb + 4
                aoh = [aop.tile([128, 4 * 65], F32, tag=f"ao{h}", bufs=1,
                                name=f"ao{h}") for h in range(2)]
                pend = []

                def flush_one(in_loop=True):
                    kb, ptt, w, j = pend.pop(0)
                    if in_loop:
                        # PV needs v1[:, kb]; the previous group's closeout
                        # must precede this group's first PV (AO reuse).
                        drain({f"v{kb}", "co"})
                    else:
                        drain({f"v{kb}"})
                    j0 = max(j, 0)
                    for h in range(2):
                        lh = 2 * hp + h
                        for qsub in range(j0, 4):
                            off = h * 512 + (qsub - j0) * 128
                            # one accumulation group per AO bank: start=True
                            # zeroes the whole 2KB zero region, so only the
                            # first matmul into the bank may set it; PSUM
                            # zeroes lazily on each address's first write
                            nc.tensor.matmul(
                                aoh[h][:, qsub * 65:qsub * 65 + 65],
                                ptt[:, off:off + 128],
                                v1[:, kb, lh * 65:(lh + 1) * 65],
                                start=(kb == 0 and qsub == 0),
                                stop=(kb == nkb - 1 and qsub == 3))

                # scores read qt[hp] cols [q0, q0+512) at every kb, and
                # kt[hp] cols [kb*128, ...) progressively
                drain({f"qkQ{hp}s{qb}"})
                for kb in range(nkb):
                    drain({f"qkK{hp}s{kb // 4}"})
                    if kb == 2:
                        # previous group's closeout must be issued before
                        # any of this group's PV matmuls touch AO buffers
                        drain({"co"})
                    j = kb - 4 * qb
                    w = 512 if j < 0 else 512 - 128 * j
                    qs = q0 + (0 if j < 0 else 128 * j)
                    # head h occupies cols [h*512, h*512+w) so every matmul
                    # output stays inside one 2KB PSUM bank
                    st = bp.tile([128, 1024], F32, tag="big", bufs=3,
                                 name="st")
                    for h in range(2):
                        nc.tensor.matmul(
                            st[:, h * 512:h * 512 + w],
                            kt[hp][h * 64:(h + 1) * 64,
                                   kb * 128:(kb + 1) * 128],
                            qt[hp][h * 64:(h + 1) * 64, qs:q0 + 512],
                            start=True, stop=True)
                    ptt = wkp.tile([128, 1024], BF16, tag="pt", bufs=8,
                                   name="pt")
                    nc.scalar.activation(
                        ptt[:].rearrange("p (a b) -> p a b", a=2)[:, :, 0:w],
                        st[:].rearrange("p (a b) -> p a b", a=2)[:, :, 0:w],
                        ExpF, scale=0.125)
                    if j >= 0:
                        for h in range(2):
                            with nc.allow_low_precision(
                                    reason="0/1 mask multiply"):
                                nc.gpsimd.tensor_mul(
                                    ptt[:, h * 512:h * 512 + 128],
                                    ptt[:, h * 512:h * 512 + 128], tri[:])
                    pend.append((kb, ptt, w, j))
                    npv = 2 * (4 - max(j, 0))
                    if len(pend) > trail:
                        flush_one()
                    # deficit: exp time minus this kb's attention PE time
                    act_ns = 2 * w * 0.8333 + 220
                    pe_ns = 2 * w * 0.4167 + npv * 30
                    pump(act_ns - pe_ns)

                def do_norm():
                    # normalize into [q, dh-pair] SBUF tiles, then XBAR-DMA
                    # transpose each back into ot[hp][:, q block]
                    rc = wkp.tile([128, 8], F32, tag="rcp", bufs=2,
                                  name="rc")
                    for h in range(2):
                        with nc.allow_low_precision(
                                reason="softmax denom recip"):
                            nc.vector.reciprocal(
                                rc[:].rearrange("p (h q) -> p h q",
                                                h=2)[:, h, :].rearrange(
                                                    "p (q c) -> p q c", c=1),
                                aoh[h].rearrange("p (q c) -> p q c",
                                                 c=65)[:, :, 64:65])
                    for qsub in range(4):
                        asb = wkp.tile([128, 128], BF16, tag="aosb", bufs=8,
                                       name="asb")
                        for h in range(2):
                            with nc.allow_low_precision(
                                    reason="bf16 attn out"):
                                if norm_on_act:
                                    # post-exp-stream groups: Act is idle
                                    nc.scalar.mul(
                                        asb[:, h * 64:(h + 1) * 64],
                                        aoh[h][:, qsub * 65:qsub * 65 + 64],
                                        rc[:, h * 4 + qsub:h * 4 + qsub + 1])
                                else:
                                    nc.vector.tensor_scalar_mul(
                                        asb[:, h * 64:(h + 1) * 64],
                                        aoh[h][:, qsub * 65:qsub * 65 + 64],
                                        rc[:, h * 4 + qsub:h * 4 + qsub + 1])
                        nc.sync.dma_start_transpose(
                            ot[hp][:, q0 + qsub * 128:q0 + (qsub + 1) * 128],
                            asb[:])

                ntail = len(pend)

                def do_closeout():
                    while pend:
                        flush_one(in_loop=False)
                    do_norm()

                return ("co", ntail * 240.0, do_closeout)

            # ---- schedule ----
            # upfront: just enough projection to start attention (0,0);
            # everything else becomes filler, force-drained on first use
            qk_chunk(0, 0, 0)
            qk_chunk(1, 0, 0)

            QK_NS = 8 * 512 * 0.4167
            V_NS = 8 * 256 * 0.4167
            OP_NS = 4 * 512 * 0.4167

            def add_qk(m, sb):
                fillers.append((f"qkQ{m}s{sb}", QK_NS,
                                lambda sb=sb, m=m: qk_chunk(0, m, sb)))
                fillers.append((f"qkK{m}s{sb}", QK_NS,
                                lambda sb=sb, m=m: qk_chunk(1, m, sb)))

            def add_v(lo, hi):
                for st in range(lo, hi):
                    fillers.append((f"v{st}", V_NS,
                                    lambda st=st: v_chunk(st)))

            # deadline order: each entry no later than its force-drain point
            add_qk(0, 1)
            add_v(0, 4)
            add_qk(0, 2)
            add_v(4, 8)
            add_qk(1, 0)
            add_v(8, 12)
            add_qk(0, 3)
            add_v(12, 16)
            add_qk(1, 1)
            add_qk(1, 2)
            add_qk(1, 3)

            groups = [(0, 0), (1, 0), (2, 0), (3, 0), (0, 1), (1, 1),
                      (2, 1), (3, 1)]
            for gi, (qb, hp) in enumerate(groups):
                co = attn_group(qb, hp, trail=2 if gi == 7 else 4,
                                norm_on_act=False)
                fillers.insert(0, co)
                if hp == 1:
                    for ei, et in enumerate(range(0, NKT, 2)):
                        # past the exp stream's end, Act is idle: alternate
                        # the PSUM->SBUF copies between Act and DVE
                        on_act = gi >= 6 and ei % 2 == 1
                        fillers.append(
                            (f"op{qb}", OP_NS,
                             lambda qb=qb, et=et, a=on_act:
                             oproj_pair(qb, et, a)))
            drain({t for t, _, _ in fillers})

    nc.compile()
    return nc


_NC = None


def _get_nc():
    global _NC
    if _NC is None:
        _NC = build_nc()
    return _NC


def make_in_maps(inputs, Wq, bq, Wk, bk, Wv, Wo):
    kk = np.arange(128)[:, None]
    qq = np.arange(128)[None, :]
    tri = (qq >= kk).astype(NPBF16)
    in_maps = []
    for c in range(NCORES):
        b, g = c // HPC, c % HPC
        sl = slice(g * DL, (g + 1) * DL)
        bqk = np.stack([bq[sl][:128], bq[sl][128:],
                        bk[sl][:128], bk[sl][128:]], axis=1)
        in_maps.append({
            "xt": np.ascontiguousarray(inputs[b].T).astype(NPBF16),
            "wq": np.ascontiguousarray(Wq[:, sl]).astype(NPBF16),
            "wk": np.ascontiguousarray(Wk[:, sl]).astype(NPBF16),
            "wv": np.ascontiguousarray(Wv[:, sl]).astype(NPBF16),
            "wo": np.ascontiguousarray(Wo[sl, :]).astype(NPBF16),
            "bqk": np.ascontiguousarray(bqk).astype(np.float32),
            "tri": tri,
        })
    return in_maps


def kernel(inputs, Wq, bq, Wk, bk, Wv, bv, Wo, bo):
    inputs = np.asarray(inputs, np.float32)
    Wq, bq, Wk, bk, Wv, bv, Wo, bo = (
        np.asarray(a, np.float32) for a in (Wq, bq, Wk, bk, Wv, bv, Wo, bo))
    in_maps = make_in_maps(inputs, Wq, bq, Wk, bk, Wv, Wo)
    nc = _get_nc()
    res = run_bass_kernel_spmd(nc, in_maps, list(range(NCORES)))
    bo_eff = bo + bv @ Wo  # V bias commutes through softmax (weights sum to 1)
    outs = []
    for b in range(B):
        acc = res.results[b * HPC]["out"].astype(np.float32)
        for g in range(1, HPC):
            acc = acc + res.results[b * HPC + g]["out"].astype(np.float32)
        outs.append(acc.T + bo_eff)
    return np.stack(outs).astype(np.float32)
